# revision 1
# baseline (speedup 1.0000x reference)
"""Bilateral filter (nn_BilateralFilter) on 8 Trainium2 NeuronCores.

Sharding: data-parallel over (batch, H-half): core i -> sample i//2,
row-half i%2 (128 output rows each). Each core receives a host-padded
input slab [C, 132, 260] (2-row/2-col zero halos) plus per-sample tap
weights derived from `params` on the host; it computes the 5x5 (or
masked 3x3) bilateral filter for its 128x256 output tile.

Math (exact rewrite of the reference):
  out[c,p] = sum_t sk[t]*e_t[p]*x[c,p+t] / sum_t (sk[t]+1e-8*mask[t])*e_t[p]
  e_t[p]   = exp(-((m*s)[p+t] - (m*s)[p])^2),  s = 1/(sqrt(2)*sigma2)
where m is the channel-mean image and sk is the mask-folded normalized
spatial kernel. The 1e-8*mask term reproduces the reference's
`w/(w.sum()+1e-8)` epsilon after multiplying through by the color-kernel
normalizer.

Engine split: per-pixel weights rule out TensorE (no shared operand), so
the 25-tap MAC stream is elementwise; channels [0:C0] run on VectorE and
[C0:C] on GPSIMD concurrently. ScalarE computes the Square/Exp chain.
"""

import numpy as np

B, C, H, W = 4, 32, 256, 256
HALF = H // 2          # output rows per core
SLAB_H = HALF + 4      # input rows incl. 2-row halos
SLAB_W = W + 4         # input cols incl. 2-col halos
NCORES = 8
NT = 25                # 5x5 taps
C0 = 22                # channels on VectorE; rest on GPSIMD

_CACHE = {}


def _host_tap_constants(params):
    """Per-sample sk_eff[25], sk2[25], s2c scalar (all float32 math)."""
    p = params.astype(np.float32)
    sig = (1.0 / (1.0 + np.exp(-p))).astype(np.float32)
    coords = (np.arange(5, dtype=np.float32) - 2.0)
    grid = coords[:, None] ** 2 + coords[None, :] ** 2
    center3 = ((np.abs(coords)[:, None] <= 1) & (np.abs(coords)[None, :] <= 1)).astype(np.float32)
    out = []
    for b in range(B):
        k_raw = np.float32(1.0) + np.float32(2.0) * sig[b, 0]
        is5 = bool(k_raw >= 2.0)
        sigma1 = np.float32(3.5) + np.float32(5.5) * sig[b, 1]
        sigma2 = np.float32(5.5) + np.float32(7.5) * sig[b, 2]
        mask = np.ones((5, 5), np.float32) if is5 else center3
        sk = np.exp(-grid / (2.0 * sigma1 ** 2)).astype(np.float32) * mask
        sk = (sk / sk.sum()).astype(np.float32)
        sk_eff = sk.reshape(NT)
        sk2 = (sk_eff + np.float32(1e-8) * mask.reshape(NT)).astype(np.float32)
        # m_s = (sum_c x) * s2c  ==  mean * (1/(sqrt(2)*sigma2))
        s2c = np.float32(1.0 / (np.sqrt(2.0, dtype=np.float64) * float(sigma2)) / C)
        out.append((sk_eff, sk2, s2c, is5))
    return out


def _build(active_taps, n_iter=1, c0=C0):
    from contextlib import ExitStack, nullcontext
    import concourse.tile as tile
    import concourse.bass as bass
    from concourse import bacc, mybir

    f32 = mybir.dt.float32
    AF = mybir.ActivationFunctionType
    AL = mybir.AluOpType
    act = sorted(active_taps)
    t_first, t_last = act[0], act[-1]

    nc = bacc.Bacc("TRN2", target_bir_lowering=False, debug=False,
                   num_devices=NCORES)
    xs_d = nc.dram_tensor("xs", [C, SLAB_H, SLAB_W], f32, kind="ExternalInput").ap()
    cst_d = nc.dram_tensor("cst", [128, 51], f32, kind="ExternalInput").ap()
    id_d = nc.dram_tensor("ident", [128, 128], f32, kind="ExternalInput").ap()
    id4_d = nc.dram_tensor("ident4", [128, 4], f32, kind="ExternalInput").ap()
    out_d = nc.dram_tensor("out", [C, HALF, W], f32, kind="ExternalOutput").ap()

    HC = C // 2  # channels per PSUM half
    NB = 8       # PSUM banks per half
    BC = HC // NB  # channels per bank (2)

    with tile.TileContext(nc) as tc, ExitStack() as ctx:
        loop_ctx = tc.For_i(0, n_iter, 1) if n_iter > 1 else nullcontext()
        pool_c = ctx.enter_context(tc.tile_pool(name="cstp", bufs=1))
        pool_x = ctx.enter_context(tc.tile_pool(name="xp", bufs=2))
        pool_mean = ctx.enter_context(tc.tile_pool(name="meanp", bufs=1))
        pool_w = ctx.enter_context(tc.tile_pool(name="wp", bufs=2))
        pool_wp = ctx.enter_context(tc.tile_pool(name="wplanes", bufs=1))
        pool_tmp = ctx.enter_context(tc.tile_pool(name="tmpp", bufs=2))
        pool_ps = ctx.enter_context(
            tc.tile_pool(name="psum", bufs=1, space=bass.MemorySpace.PSUM))

        cst = pool_c.tile([128, 51], f32, name="cst")
        nc.sync.dma_start(cst[:], cst_d)
        ident = pool_c.tile([128, 128], f32, name="ident")
        nc.sync.dma_start(ident[:], id_d)
        ident4 = pool_c.tile([128, 4], f32, name="ident4")
        nc.sync.dma_start(ident4[:], id4_d)
        ctx.enter_context(loop_ctx)

        # ---- x slab rows 0..128 (mean) + tail rows 128..132 ----
        xg0 = pool_mean.tile([128, C, SLAB_W], f32, name="xg0")
        nc.sync.dma_start(xg0[:], xs_d[:, 0:128, :].transpose([1, 0, 2]))
        xt = pool_mean.tile([128, SLAB_W], f32, name="xt")
        for r in range(4):
            nc.sync.dma_start(xt[r * 32:(r + 1) * 32, :], xs_d[:, 128 + r, :])

        # ---- channel sum on PE (identity-matmul accumulate into PSUM) ----
        ps_m = pool_ps.tile([128, SLAB_W], f32, name="ps_m")
        ps_mt = pool_ps.tile([4, SLAB_W], f32, name="ps_mt")
        for c in range(C):
            nc.tensor.matmul(ps_m[:], ident[:], xg0[:, c, :],
                             start=(c == 0), stop=(c == C - 1))
        nc.tensor.matmul(ps_mt[:], ident4[:], xt[:], start=True, stop=True)

        # scaled mean m_s = (sum_c x) * s2c   (cst col 50)
        m_sA = pool_mean.tile([128, SLAB_W], f32, name="m_sA")
        nc.vector.tensor_scalar_mul(out=m_sA[:], in0=ps_m[:], scalar1=cst[:, 50:51])
        m_sB = pool_mean.tile([4, SLAB_W], f32, name="m_sB")
        nc.vector.tensor_scalar_mul(out=m_sB[:], in0=ps_mt[:], scalar1=cst[0:4, 50:51])

        # di-shifted views of m_s (rows di..di+128 of the slab)
        msd = {0: m_sA}
        for di in range(1, 5):
            t = pool_mean.tile([128, SLAB_W], f32, tag=f"msd{di}", name=f"msd{di}")
            nc.sync.dma_start(t[0:128 - di, :], m_sA[di:128, :])
            nc.sync.dma_start(t[128 - di:128, :], m_sB[0:di, :])
            msd[di] = t

        # ---- phase A: per-tap weight planes + denominator (DVE + ACT) ----
        # Batched over the 5 dj taps of each di via overlapping-window APs.
        Wp = {}
        denom = pool_mean.tile([128, W], f32, name="denom")
        act_dis = sorted({t // 5 for t in act})
        first_di = act_dis[0]
        for di in act_dis:
            djs = [dj for dj in range(5) if (di * 5 + dj) in active_taps]
            dj0, ndj = djs[0], len(djs)
            # d5[p, j, w] = m_s_di[p, w+dj0+j] - m_s_c[p, w]: overlapping
            # windows along w expressed as a hand-built 3D AP (dj step 1).
            from concourse.ap import AP as _AP
            in0 = msd[di][:, dj0:dj0 + W + ndj - 1]
            in0w = _AP(in0.tensor, in0.offset,
                       [list(in0.ap[0]), [1, ndj], [1, W]])
            ctr = msd[2][:, 2:2 + W].unsqueeze(1).broadcast_to([128, ndj, W])
            d5 = pool_w.tile([128, ndj, W], f32, tag="d", name=f"d{di}")
            nc.vector.tensor_tensor(out=d5[:], in0=in0w, in1=ctr, op=AL.subtract)
            sq5 = pool_w.tile([128, ndj, W], f32, tag="sq", name=f"sq{di}")
            nc.scalar.activation(out=sq5[:], in_=d5[:], func=AF.Square)
            e5 = pool_w.tile([128, ndj, W], f32, tag="e", name=f"e{di}")
            nc.scalar.activation(out=e5[:], in_=sq5[:], func=AF.Exp, scale=-1.0)
            # W planes: one tile per di holding ndj planes; sk broadcast per tap
            W5 = pool_wp.tile([128, ndj, W], f32, tag=f"W{di}", bufs=1, name=f"W{di}")
            for j, dj in enumerate(djs):
                t_idx = di * 5 + dj
                nc.scalar.mul(W5[:, j, :], e5[:, j, :], cst[:, t_idx:t_idx + 1])
            for j, dj in enumerate(djs):
                Wp[di * 5 + dj] = W5[:, j, :]
            # denom chain per tap (scalar per-partition STT)
            for j, dj in enumerate(djs):
                t_idx = di * 5 + dj
                if t_idx == t_first:
                    nc.vector.tensor_scalar_mul(
                        out=denom[:], in0=e5[:, j, :],
                        scalar1=cst[:, 25 + t_idx:26 + t_idx])
                else:
                    nc.vector.scalar_tensor_tensor(
                        out=denom[:], in0=e5[:, j, :],
                        scalar=cst[:, 25 + t_idx:26 + t_idx],
                        in1=denom[:], op0=AL.mult, op1=AL.add)
        recip = pool_mean.tile([128, W], f32, name="recip")
        nc.vector.reciprocal(out=recip[:], in_=denom[:])

        # ---- phase B: per-half MAC: DVE mults; adds split PE(psum)/DVE ----
        QP = 12           # channels accumulated on PE per half (6 banks)
        QD = HC - QP      # channels accumulated on DVE per half
        for half in range(2):
            ch0 = half * HC
            pb = [pool_ps.tile([128, BC, W], f32, tag=f"pb{b}", name=f"pb{half}_{b}")
                  for b in range(QP // BC)]
            acc_sb = pool_tmp.tile([128, QD, W], f32, tag="acc_sb", bufs=1, name=f"accs{half}")
            for di in range(5):
                if not any((di * 5 + dj) in active_taps for dj in range(5)):
                    continue
                xh = pool_x.tile([128, HC, SLAB_W], f32, tag="xh", name=f"xh{half}_{di}")
                nc.sync.dma_start(
                    xh[:], xs_d[ch0:ch0 + HC, di:di + 128, :].transpose([1, 0, 2]))
                for dj in range(5):
                    t_idx = di * 5 + dj
                    if t_idx not in active_taps:
                        continue
                    prod = pool_tmp.tile([128, HC, W], f32, tag="prod", bufs=3,
                                         name=f"pr{half}_{t_idx}")
                    nc.vector.tensor_tensor(
                        out=prod[:],
                        in0=Wp[t_idx].unsqueeze(1).broadcast_to([128, HC, W]),
                        in1=xh[:, :, dj:dj + W], op=AL.mult)
                    for b in range(QP // BC):
                        nc.tensor.matmul(
                            pb[b][:], ident[:], prod[:, b * BC:(b + 1) * BC, :],
                            start=(t_idx == t_first), stop=(t_idx == t_last))
                    if t_idx == t_first:
                        nc.vector.tensor_copy(acc_sb[:], prod[:, QP:HC, :])
                    else:
                        nc.vector.tensor_add(acc_sb[:], acc_sb[:], prod[:, QP:HC, :])
            og = pool_tmp.tile([128, HC, W], f32, tag="prod", bufs=3, name=f"og{half}")
            for b in range(QP // BC):
                nc.vector.tensor_tensor(
                    out=og[:, b * BC:(b + 1) * BC, :], in0=pb[b][:],
                    in1=recip[:].unsqueeze(1).broadcast_to([128, BC, W]),
                    op=AL.mult)
            nc.vector.tensor_tensor(
                out=og[:, QP:HC, :], in0=acc_sb[:],
                in1=recip[:].unsqueeze(1).broadcast_to([128, QD, W]), op=AL.mult)
            nc.sync.dma_start(
                out_d[ch0:ch0 + HC, :, :].transpose([1, 0, 2]), og[:])

    nc.compile()
    return nc


def _prep_inputs(x, params):
    """Build per-core in_maps."""
    x = np.ascontiguousarray(x, dtype=np.float32)
    tap_consts = _host_tap_constants(params)
    active = set()
    for (sk_eff, sk2, s2c, is5) in tap_consts:
        active |= {t for t in range(NT) if sk2[t] != 0.0}
    # pad whole batch once: [B, C, H+4, W+4]
    xp = np.pad(x, ((0, 0), (0, 0), (2, 2), (2, 2)))
    in_maps = []
    for core in range(NCORES):
        b, half = core // 2, core % 2
        h0 = half * HALF
        slab = np.ascontiguousarray(xp[b, :, h0:h0 + SLAB_H, :])
        sk_eff, sk2, s2c, _ = tap_consts[b]
        cst = np.zeros((128, 51), np.float32)
        cst[:, 0:25] = sk_eff[None, :]
        cst[:, 25:50] = sk2[None, :]
        cst[:, 50] = s2c
        in_maps.append({"xs": slab, "cst": cst,
                        "ident": np.eye(128, dtype=np.float32),
                        "ident4": np.repeat(np.eye(4, dtype=np.float32), 32, axis=0)})
    return in_maps, frozenset(active)


def kernel(x, params, n_iter=1, c0=C0):
    from concourse.bass_utils import run_bass_kernel_spmd
    in_maps, active = _prep_inputs(x, params)
    key = ("nc", active, n_iter, c0)
    if key not in _CACHE:
        _CACHE[key] = _build(active, n_iter, c0)
    nc = _CACHE[key]
    res = run_bass_kernel_spmd(nc, in_maps, list(range(NCORES)))
    out = np.empty((B, C, H, W), np.float32)
    for core in range(NCORES):
        b, half = core // 2, core % 2
        out[b, :, half * HALF:(half + 1) * HALF, :] = res.results[core]["out"]
    return out



# revision 21
# speedup vs baseline: 1.5170x; 1.5170x over previous
"""Bilateral filter (nn_BilateralFilter) on 8 Trainium2 NeuronCores.

Sharding: data-parallel over (batch, H-half): core i -> sample i//2,
row-half i%2 (128 output rows each). Each core receives a host-padded
input slab [C, 132, 260] (2-row/2-col zero halos) plus per-sample tap
constants; it computes the 5x5 (or masked 3x3) bilateral filter for its
128x256 output tile.

Math (exact rewrite of the reference):
  out[c,p] = sum_t W_t[p] x[c,p+t] / (sum_t W_t[p] + 1e-8*n_active)
  W_t[p]   = exp(-((m*s)[p+t] - (m*s)[p])^2 + ln sk_t),
  s = 1/(sqrt(2)*sigma2*C), sk = mask-folded normalized spatial kernel.
The 1e-8*n_active term reproduces the reference's w/(w.sum()+1e-8)
epsilon after multiplying through by the color-kernel normalizer
(sum_t e_t ~= n_active since e_t in [0.96, 1]; abs err <= 1e-8*25*0.04).
Inactive union taps get ln_sk = -100 so their weight underflows to 0.

Engine split: tap multiplies in bf16 on DVE (2x mode, dj-paired APs;
~16 pair-ops) and GPSIMD (~18 single-tap ops); tap accumulation via
bf16 identity matmuls into PSUM on PE (1 cyc/row); weights on ACT as a
single Exp(-d^2 + ln sk) per tap (bf16 out); denominator as a DVE add
chain over W planes; final psum*recip on GPSIMD. Row-shifted x slabs
come from a bf16 restage of the slab in DRAM (avoids per-partition-fat
SBUF tail copies); all DRAM-touching DMAs stay on the SP queue in
dependency order.
"""

import numpy as np

B, C, H, W = 4, 32, 256, 256
HALF = H // 2          # output rows per core
SLAB_H = HALF + 4      # input rows incl. 2-row halos
SLAB_W = W + 4         # input cols incl. 2-col halos
NCORES = 8
NT = 25                # 5x5 taps
HC = C // 2            # channels per PSUM group
CHK = 4                # channels per fp32 staging chunk

_CACHE = {}


def _host_tap_constants(params):
    """Per-sample ln_sk[25], n_active, s2c scalar (all float32 math)."""
    p = params.astype(np.float32)
    sig = (1.0 / (1.0 + np.exp(-p))).astype(np.float32)
    coords = (np.arange(5, dtype=np.float32) - 2.0)
    grid = coords[:, None] ** 2 + coords[None, :] ** 2
    center3 = ((np.abs(coords)[:, None] <= 1) & (np.abs(coords)[None, :] <= 1)).astype(np.float32)
    out = []
    for b in range(B):
        k_raw = np.float32(1.0) + np.float32(2.0) * sig[b, 0]
        is5 = bool(k_raw >= 2.0)
        sigma1 = np.float32(3.5) + np.float32(5.5) * sig[b, 1]
        sigma2 = np.float32(5.5) + np.float32(7.5) * sig[b, 2]
        mask = np.ones((5, 5), np.float32) if is5 else center3
        sk = np.exp(-grid / (2.0 * sigma1 ** 2)).astype(np.float32) * mask
        sk = (sk / sk.sum()).astype(np.float32)
        sk_eff = sk.reshape(NT)
        active = sk_eff > 0.0
        ln_sk = np.where(active, np.log(np.maximum(sk_eff, 1e-30)),
                         np.float32(-100.0)).astype(np.float32)
        n_act = np.float32(active.sum())
        s2c = np.float32(1.0 / (np.sqrt(2.0, dtype=np.float64) * float(sigma2)) / C)
        out.append((ln_sk, n_act, s2c, active))
    return out


def _build(active_taps, n_iter=1):
    from contextlib import ExitStack, nullcontext
    import concourse.tile as tile
    import concourse.bass as bass
    from concourse import bacc, mybir
    from concourse.ap import AP as _AP

    f32 = mybir.dt.float32
    bf16 = mybir.dt.bfloat16
    AF = mybir.ActivationFunctionType
    AL = mybir.AluOpType
    act = sorted(active_taps)
    act_dis = sorted({t // 5 for t in act})

    nc = bacc.Bacc("TRN2", target_bir_lowering=False, debug=False,
                   num_devices=NCORES)
    xs_d = nc.dram_tensor("xs", [C, SLAB_H, SLAB_W], f32, kind="ExternalInput").ap()
    cst_d = nc.dram_tensor("cst", [128, 64], f32, kind="ExternalInput").ap()
    id_d = nc.dram_tensor("ident", [128, 128], f32, kind="ExternalInput").ap()
    id4_d = nc.dram_tensor("ident4", [128, 4], f32, kind="ExternalInput").ap()
    xbf_d = nc.dram_tensor("xbf", [C, SLAB_H, SLAB_W], bf16, kind="Internal").ap()
    out_d = nc.dram_tensor("out", [C, HALF, W], f32, kind="ExternalOutput").ap()

    # Per-(di) engine split of dj taps: even di -> pair up djs on DVE with
    # one leftover single on GPSIMD; odd di -> one pair on DVE, rest GPSIMD.
    def tap_plan(di, djs):
        npair = len(djs) // 2 if di % 2 == 0 else max(0, len(djs) // 2 - 1)
        pairs, singles, i = [], [], 0
        while npair > 0 and i + 1 < len(djs):
            if djs[i + 1] == djs[i] + 1:
                pairs.append((djs[i], djs[i + 1]))
                i += 2
                npair -= 1
            else:
                singles.append(djs[i])
                i += 1
        singles += djs[i:]
        return pairs, singles

    with tile.TileContext(nc) as tc, ExitStack() as ctx:
        loop_ctx = tc.For_i(0, n_iter, 1) if n_iter > 1 else nullcontext()
        pool_c = ctx.enter_context(tc.tile_pool(name="cstp", bufs=1))
        pool_s = ctx.enter_context(tc.tile_pool(name="stagep", bufs=2))
        pool_x = ctx.enter_context(tc.tile_pool(name="xp", bufs=1))
        pool_m = ctx.enter_context(tc.tile_pool(name="meanp", bufs=1))
        pool_w = ctx.enter_context(tc.tile_pool(name="wp", bufs=1))
        pool_pd = ctx.enter_context(tc.tile_pool(name="prodpd", bufs=2))
        pool_pg = ctx.enter_context(tc.tile_pool(name="prodpg", bufs=3))
        pool_o = ctx.enter_context(tc.tile_pool(name="ogp", bufs=1))
        pool_ps = ctx.enter_context(
            tc.tile_pool(name="psum", bufs=1, space=bass.MemorySpace.PSUM))

        cst = pool_c.tile([128, 64], f32, name="cst")
        nc.sync.dma_start(cst[:], cst_d)
        ident = pool_c.tile([128, 128], f32, name="ident")
        nc.sync.dma_start(ident[:], id_d)
        ident4 = pool_c.tile([128, 4], f32, name="ident4")
        nc.sync.dma_start(ident4[:], id4_d)
        identb = pool_c.tile([128, 128], bf16, name="identb")
        nc.scalar.activation(out=identb[:], in_=ident[:], func=AF.Copy)
        id4b = pool_c.tile([128, 4], bf16, name="id4b")
        nc.scalar.activation(out=id4b[:], in_=ident4[:], func=AF.Copy)
        ctx.enter_context(loop_ctx)

        # ---- chunked fp32 load -> bf16 convert -> channel-sum (PE) ----
        # tail rows 128..132 packed c-major: partition c*4+r, loaded first
        # one untransposed DMA: flat (c, r, w) enumeration lands on
        # partition c*4+r -- c-major packing for free
        xtf = pool_s.tile([128, SLAB_W], f32, name="xtf")
        nc.sync.dma_start(xtf[:], xs_d[:, 128:132, :])
        xtb = pool_x.tile([128, SLAB_W], bf16, name="xtb")
        nc.scalar.activation(out=xtb[:], in_=xtf[:], func=AF.Copy)
        # untransposed tail restage write: dst [C, 4, 260] <- c-major src
        nc.sync.dma_start(xbf_d[:, 128:132, :], xtb[:])
        ps_t = pool_ps.tile([128, 2, W], f32, tag="bk1", name="ps_t")
        ps_tA = _AP(ps_t[:].tensor, ps_t[:].offset,
                    [[ps_t[:].ap[0][0], 4], [1, SLAB_W]])
        nc.tensor.matmul(ps_tA, id4b[:], xtb[:], start=True, stop=True,
                         skip_group_check=True)
        m_sB = pool_m.tile([4, SLAB_W], f32, name="m_sB")
        nc.vector.tensor_scalar_mul(
            out=m_sB[:],
            in0=_AP(ps_t[:].tensor, ps_t[:].offset,
                    [[ps_t[:].ap[0][0], 4], [1, SLAB_W]]),
            scalar1=cst[0:4, 51:52])

        xb = {0: pool_x.tile([128, C, SLAB_W], bf16, name="xb0")}
        ps_m = pool_ps.tile([128, 2, W], f32, tag="bk0", name="ps_m")
        ps_mA = _AP(ps_m[:].tensor, ps_m[:].offset,
                    [list(ps_m[:].ap[0]), [1, SLAB_W]])
        for k in range(C // CHK):
            xst = pool_s.tile([128, CHK, SLAB_W], f32, tag="xst", name=f"xst{k}")
            nc.sync.dma_start(
                xst[:], xs_d[k * CHK:(k + 1) * CHK, 0:128, :].transpose([1, 0, 2]))
            nc.scalar.activation(out=xb[0][:, k * CHK:(k + 1) * CHK, :],
                                 in_=xst[:], func=AF.Copy)
            for c in range(CHK):
                cc = k * CHK + c
                nc.tensor.matmul(ps_mA, identb[:], xb[0][:, cc, :],
                                 start=(cc == 0), stop=(cc == C - 1),
                                 skip_group_check=True)

        # scaled mean m_s = (sum_c x) * s2c   (cst col 51)
        m_sA = pool_m.tile([128, SLAB_W], f32, name="m_sA")
        nc.vector.tensor_scalar_mul(out=m_sA[:], in0=ps_mA, scalar1=cst[:, 51:52])

        # di-shifted views of m_s (rows di..di+128); tails (from early m_sB)
        # first, then mains; msd2 first (every di subtract needs the center)
        msd = {0: m_sA}
        for di in (2, 1, 3, 4):
            t = pool_m.tile([128, SLAB_W], f32, name=f"msd{di}")
            nc.sync.dma_start(t[128 - di:128, :], m_sB[0:di, :])
            msd[di] = t
        for di in (2, 1, 3, 4):
            nc.sync.dma_start(msd[di][0:128 - di, :], m_sA[di:128, :])

        # ---- restage bf16 slab rows 0..128 to DRAM (after msd DMAs) ----
        for k in range(C // CHK):
            nc.sync.dma_start(
                xbf_d[k * CHK:(k + 1) * CHK, 0:128, :].transpose([1, 0, 2]),
                xb[0][:, k * CHK:(k + 1) * CHK, :])

        # ---- shifted bf16 slabs from restaged DRAM ----
        for di in act_dis:
            if di == 0:
                continue
            t = pool_x.tile([128, C, SLAB_W], bf16, name=f"xb{di}")
            nc.sync.dma_start(t[:], xbf_d[:, di:di + 128, :].transpose([1, 0, 2]))
            xb[di] = t

        # ---- phase A: weight planes W_t = Exp(-d^2 + ln sk_t) in bf16 ----
        W5 = pool_c.tile([128, NT, W], bf16, name="W5")
        denom = pool_m.tile([128, W], f32, name="denom")
        first_t = act[0]
        for di in act_dis:
            djs = [dj for dj in range(5) if (di * 5 + dj) in active_taps]
            dj0, ndj = djs[0], len(djs)
            in0 = msd[di][:, dj0:dj0 + W + ndj - 1]
            in0w = _AP(in0.tensor, in0.offset,
                       [list(in0.ap[0]), [1, ndj], [1, W]])
            ctr = msd[2][:, 2:2 + W].unsqueeze(1).broadcast_to([128, ndj, W])
            d5 = pool_w.tile([128, ndj, W], f32, tag="d", name=f"d{di}")
            nc.gpsimd.tensor_tensor(out=d5[:], in0=in0w, in1=ctr, op=AL.subtract)
            sq5 = pool_w.tile([128, ndj, W], f32, tag="sq", name=f"sq{di}")
            nc.scalar.activation(out=sq5[:], in_=d5[:], func=AF.Square)
            for j, dj in enumerate(djs):
                t_idx = di * 5 + dj
                nc.scalar.activation(out=W5[:, t_idx, :], in_=sq5[:, j, :],
                                     func=AF.Exp, scale=-1.0,
                                     bias=cst[:, t_idx:t_idx + 1])
                # denominator: sum_t W_t as a GPSIMD chain (SBUF only)
                if t_idx == first_t:
                    nc.gpsimd.tensor_copy(denom[:], W5[:, t_idx, :])
                else:
                    nc.gpsimd.tensor_tensor(out=denom[:], in0=W5[:, t_idx, :],
                                            in1=denom[:], op=AL.add)
        # recip = 1 / (sum_t W_t + 1e-8*n_active)   (cst col 50)
        dsum = pool_m.tile([128, W], f32, name="dsum")
        nc.vector.tensor_scalar_add(out=dsum[:], in0=denom[:], scalar1=cst[:, 50:51])
        recip = pool_m.tile([128, W], f32, name="recip")
        nc.vector.reciprocal(out=recip[:], in_=dsum[:])

        # ---- phase B: tap MAC; 4 groups x 8ch, psum bank ping-pong.
        # Groups run in pairs (gA even banks 0-3, gB banks 4-7) sweeping di
        # together, so each shifted slab is needed at half the pace and og
        # drains overlap the next pair's matmuls. ----
        GC = 8   # channels per group
        NB = GC // 2  # psum banks per group
        n_taps = len(act)

        def emit_pair_op(g, di, pr):
            ch0 = g * GC
            t0 = di * 5 + pr[0]
            prod = pool_pd.tile([128, 2, GC, W], bf16, tag="pd",
                                name=f"pr{g}_{t0}")
            w2 = W5[:, t0:t0 + 2, :].unsqueeze(2).broadcast_to([128, 2, GC, W])
            xin = xb[di][:, ch0:ch0 + GC, pr[0]:pr[0] + W + 1]
            xw = _AP(xin.tensor, xin.offset,
                     [list(xin.ap[0]), [1, 2], list(xin.ap[1]), [1, W]])
            nc.vector.tensor_tensor(out=prod[:], in0=w2, in1=xw, op=AL.mult)
            return [prod[:, 0, :, :], prod[:, 1, :, :]]

        def emit_single_op(g, di, dj):
            ch0 = g * GC
            t_idx = di * 5 + dj
            prod = pool_pg.tile([128, GC, W], bf16, tag="pg",
                                name=f"pg{g}_{t_idx}")
            nc.gpsimd.tensor_tensor(
                out=prod[:],
                in0=W5[:, t_idx, :].unsqueeze(1).broadcast_to([128, GC, W]),
                in1=xb[di][:, ch0:ch0 + GC, dj:dj + W],
                op=AL.mult)
            return [prod[:]]

        for gA in (0, 2):
            gB = gA + 1
            pbs = {g: [pool_ps.tile([128, 2, W], f32,
                                    tag=f"bk{(g % 2) * NB + b}",
                                    name=f"pb{g}_{b}")
                       for b in range(NB)]
                   for g in (gA, gB)}
            done = {gA: 0, gB: 0}

            def accumulate(g, prods):
                for pap in prods:
                    for b in range(NB):
                        nc.tensor.matmul(pbs[g][b][:], identb[:],
                                         pap[:, 2 * b:2 * b + 2, :],
                                         start=(done[g] == 0),
                                         stop=(done[g] == n_taps - 1),
                                         skip_group_check=True)
                    done[g] += 1

            for di in act_dis:
                djs = [dj for dj in range(5) if (di * 5 + dj) in active_taps]
                pairs, singles = tap_plan(di, djs)
                # alternate which group leads on DVE vs GPSIMD per di
                lead, trail = (gA, gB) if di % 2 == 0 else (gB, gA)
                items = []
                for pr in pairs:
                    items.append(("p", lead, pr))
                    items.append(("p", trail, pr))
                for s in singles:
                    items.append(("s", lead, s))
                    items.append(("s", trail, s))
                # interleave DVE-pair ops and GPSIMD-single ops
                p_items = [it for it in items if it[0] == "p"]
                s_items = [it for it in items if it[0] == "s"]
                merged, ip, isg = [], 0, 0
                while ip < len(p_items) or isg < len(s_items):
                    if ip < len(p_items) and (
                            isg >= len(s_items)
                            or ip * max(1, len(s_items)) <=
                            isg * max(1, len(p_items))):
                        merged.append(p_items[ip]); ip += 1
                    else:
                        merged.append(s_items[isg]); isg += 1
                for kind, g, item in merged:
                    if kind == "p":
                        accumulate(g, emit_pair_op(g, di, item))
                    else:
                        accumulate(g, emit_single_op(g, di, item))

            # og = psum * recip on DVE (GPSIMD cannot read PSUM on HW)
            for g in (gA, gB):
                ch0 = g * GC
                og = pool_o.tile([128, GC, W], f32, tag="og", name=f"og{g}")
                for b in range(NB):
                    nc.vector.tensor_tensor(
                        out=og[:, 2 * b:2 * b + 2, :], in0=pbs[g][b][:],
                        in1=recip[:].unsqueeze(1).broadcast_to([128, 2, W]),
                        op=AL.mult)
                nc.sync.dma_start(
                    out_d[ch0:ch0 + GC, :, :].transpose([1, 0, 2]), og[:])

    nc.compile()
    return nc


def _prep_inputs(x, params):
    """Build per-core in_maps."""
    x = np.ascontiguousarray(x, dtype=np.float32)
    tap_consts = _host_tap_constants(params)
    active = set()
    for (ln_sk, n_act, s2c, act_mask) in tap_consts:
        active |= {t for t in range(NT) if act_mask[t]}
    xp = np.pad(x, ((0, 0), (0, 0), (2, 2), (2, 2)))
    in_maps = []
    for core in range(NCORES):
        b, half = core // 2, core % 2
        h0 = half * HALF
        slab = np.ascontiguousarray(xp[b, :, h0:h0 + SLAB_H, :])
        ln_sk, n_act, s2c, _ = tap_consts[b]
        cst = np.zeros((128, 64), np.float32)
        cst[:, 0:25] = ln_sk[None, :]
        cst[:, 50] = np.float32(1e-8) * n_act
        cst[:, 51] = s2c
        in_maps.append({"xs": slab, "cst": cst,
                        "ident": np.eye(128, dtype=np.float32),
                        "ident4": np.tile(np.eye(4, dtype=np.float32), (32, 1))})
    return in_maps, frozenset(active)


def kernel(x, params, n_iter=1, **_):
    from concourse.bass_utils import run_bass_kernel_spmd
    in_maps, active = _prep_inputs(x, params)
    key = ("nc", active, n_iter)
    if key not in _CACHE:
        _CACHE[key] = _build(active, n_iter)
    nc = _CACHE[key]
    res = run_bass_kernel_spmd(nc, in_maps, list(range(NCORES)))
    out = np.empty((B, C, H, W), np.float32)
    for core in range(NCORES):
        b, half = core // 2, core % 2
        out[b, :, half * HALF:(half + 1) * HALF, :] = res.results[core]["out"]
    return out
